# revision 1
# baseline (speedup 1.0000x reference)
"""Bezier2Image Trainium2 kernel (Bass/Tile, 8-core data parallel).

Computation per sample b:
  ctrl = x[b].reshape(160, 4, 2); pts = T @ ctrl  -> 4800 (curve, t) points
  gX[p, w] = exp(-(bX_w - X_p)^2 / ALPHA), gY likewise  (separable splat)
  out[b] = min(gX^T @ gY, 1)   (contraction over the 4800 points)

Device mapping (per core, 16 samples):
  - points are processed in 40 chunks of 120 (4 curves x 30 samples), with
    partition dim = point-within-chunk
  - pts: computed on DVE as an elementwise mul with a replicated Bernstein
    basis table followed by a reduce over the 4 control points
  - gaussians: d = k*bX - k*X via one broadcast tensor_tensor per coord,
    squared (ACT for X, DVE for Y to balance engines), exp on ACT (bf16 out)
  - accumulation: 40 bf16 matmuls [120x60]^T @ [120x60] into one PSUM bank
"""

import numpy as np

N = 30
W = 60
LENGTH = 160
ALPHA = 2e-4
B = 128
NCORES = 8
BPC = B // NCORES  # samples per core
KS = float(1.0 / np.sqrt(ALPHA))
NCH = 40  # chunks per sample
PCH = 120  # points per chunk (4 curves x 30)

_state = {}


def _bezier_T():
    t = np.arange(N, dtype=np.float64) / N
    t = 2.0 * t**3 - 3.0 * t**2 + 2.0 * t
    t3 = t**3
    T = np.stack(
        [t3, 3.0 * (t**2 - t3), 3.0 * (t3 - 2.0 * t**2 + t), (1.0 - t) ** 3],
        axis=1,
    )
    return T  # [N, 4] float64


def build_nc(loop_n=1, sim_safe=False):
    from contextlib import ExitStack

    import concourse.bacc as bacc
    import concourse.mybir as mybir
    import concourse.tile as tile

    fp32 = mybir.dt.float32
    bf16 = mybir.dt.bfloat16
    AF = mybir.ActivationFunctionType

    # Bacc (not plain Bass): its compile() pass splits multi-sem waits into
    # event-semaphore instructions — walrus codegen allows only one sync wait
    # per compute instruction.
    nc = bacc.Bacc()
    x_in = nc.declare_dram_parameter("x", [BPC, LENGTH, 8], fp32, isOutput=False)
    out_d = nc.declare_dram_parameter("out", [BPC, W, W], fp32, isOutput=True)

    # Constants.
    T = _bezier_T()  # [30, 4]
    q = np.arange(PCH)
    # Wc[(dl, k), q] = -KS * T[q % 30, k] if q // 30 == dl else 0.
    # One matmul Wc.T @ ctrl_staged then computes -KS * pts for a whole
    # sample: nkXY[q, c] = sum_{dl,k} Wc[(dl,k), q] * x[b, 4c+dl, 2k+coord].
    Wc_np = np.zeros((16, PCH), np.float32)
    for dl in range(4):
        for k in range(4):
            row = np.where(q // N == dl, -KS * T[q % N, k], 0.0)
            Wc_np[dl * 4 + k] = row.astype(np.float32)
    bxk_np = np.broadcast_to(
        (KS * np.arange(W, dtype=np.float64) / W).astype(np.float32), (128, W)
    ).copy()

    Wc_d = nc.inline_tensor(Wc_np, "Wc")
    bxk_d = nc.inline_tensor(bxk_np, "bxk")

    with ExitStack() as ctx:
        tc = ctx.enter_context(tile.TileContext(nc))
        consts = ctx.enter_context(tc.tile_pool(name="consts", bufs=1))
        small = ctx.enter_context(tc.tile_pool(name="small", bufs=4))
        big = ctx.enter_context(tc.tile_pool(name="big", bufs=6))
        psum = ctx.enter_context(tc.tile_pool(name="psum", bufs=3, space="PSUM"))
        psum_pts = ctx.enter_context(tc.tile_pool(name="psum_pts", bufs=2, space="PSUM"))
        outp = ctx.enter_context(tc.tile_pool(name="outp", bufs=6))

        Wc = consts.tile([16, PCH], fp32)
        nc.sync.dma_start(out=Wc, in_=Wc_d[:, :])
        bxk = consts.tile([128, W], fp32)
        nc.sync.dma_start(out=bxk, in_=bxk_d[:, :])

        loop_ctx = tc.For_i(0, loop_n, 1) if loop_n > 1 else None
        if loop_ctx is not None:
            ctx.enter_context(loop_ctx)

        for b in range(BPC):
            # staged[(dl,k), c, t] = x[b, 4c+dl, 2k+t]
            staged = small.tile([16, NCH, 2], fp32)
            xb = x_in[b].rearrange("(c dl) (k t) -> dl k c t", dl=4, t=2)
            for t in range(2):
                nc.sync.dma_start(
                    out=staged[:, :, t],
                    in_=xb[:, :, :, t].rearrange("dl k c -> (dl k) c"),
                )
            gs = []
            for coord in range(2):
                # nkxy[q, c] = -KS * pts[l(q,c), n(q), coord]
                nkxy = psum_pts.tile([PCH, NCH], fp32, name=f"nkxy{coord}_{b}", tag=f"nkxy{coord}")
                nc.tensor.matmul(nkxy, Wc, staged[:, :, coord])
                # ds = k*bX - k*pts, in bf16 (fine: the subtraction happens in
                # f32 before rounding; bf16 d only perturbs exp args by <<1%).
                ds = big.tile([PCH, NCH, W], bf16, name=f"ds{coord}_{b}", tag=f"ds{coord}")
                if coord == 0:
                    nc.vector.tensor_add(
                        ds,
                        bxk[:PCH].unsqueeze(1).broadcast_to([PCH, NCH, W]),
                        nkxy.unsqueeze(2).broadcast_to([PCH, NCH, W]),
                    )
                else:
                    # Split between DVE and GPSIMD for engine balance
                    # (GPSIMD cannot read PSUM: bounce via SBUF).
                    nkxy_sb = small.tile([PCH, NCH], fp32, name=f"nkxysb_{b}", tag="nkxy_sb")
                    nc.scalar.copy(nkxy_sb, nkxy)
                    cs = 12  # chunks handled by DVE
                    nc.vector.tensor_add(
                        ds[:, :cs],
                        bxk[:PCH].unsqueeze(1).broadcast_to([PCH, cs, W]),
                        nkxy[:, :cs].unsqueeze(2).broadcast_to([PCH, cs, W]),
                    )
                    nc.gpsimd.tensor_add(
                        ds[:, cs:],
                        bxk[:PCH].unsqueeze(1).broadcast_to([PCH, NCH - cs, W]),
                        nkxy_sb[:, cs:].unsqueeze(2).broadcast_to([PCH, NCH - cs, W]),
                    )
                # One ACT pass computes the gaussian directly:
                # Derivative_Erf(x) = (2/sqrt(pi)) * exp(-x^2).
                # The (4/pi) factor on gX*gY is undone in the epilogue.
                g = big.tile([PCH, NCH, W], bf16, name=f"g{coord}_{b}", tag=f"g{coord}")
                if sim_safe:
                    # CoreSim lacks Derivative_Erf: equivalent two-op path.
                    d2 = big.tile([PCH, NCH, W], bf16, name=f"d2{coord}_{b}", tag=f"d2{coord}")
                    nc.vector.tensor_mul(d2, ds, ds)
                    nc.scalar.activation(g, d2, AF.Exp, scale=-1.0)
                    nc.vector.tensor_scalar_mul(g, g, float(2.0 / np.sqrt(np.pi)))
                else:
                    nc.scalar.activation(g, ds, AF.Derivative_Erf)
                gs.append(g)

            res = psum.tile([W, W], fp32)
            for c in range(NCH):
                nc.tensor.matmul(
                    res,
                    gs[0][:, c, :],
                    gs[1][:, c, :],
                    start=(c == 0),
                    stop=(c == NCH - 1),
                )

            res_sb = outp.tile([W, W], fp32, name=f"rs_{b}", tag="res_sb")
            # res carries the (2/sqrt(pi))^2 factor from Derivative_Erf:
            # undo with *pi/4, then clamp.
            nc.vector.tensor_scalar(
                res_sb,
                res,
                float(np.pi / 4.0),
                1.0,
                op0=mybir.AluOpType.mult,
                op1=mybir.AluOpType.min,
            )
            nc.sync.dma_start(out=out_d[b], in_=res_sb)

    nc.compile()
    return nc


def kernel(x):
    import os

    x = np.ascontiguousarray(x, dtype=np.float32)
    assert x.shape == (B, LENGTH, 8), x.shape
    if "nc" not in _state:
        _state["nc"] = build_nc()
    from concourse.bass_utils import run_bass_kernel_spmd

    in_maps = [{"x": x[i * BPC : (i + 1) * BPC]} for i in range(NCORES)]
    trace = bool(os.environ.get("BEZIER_TRACE"))
    res = run_bass_kernel_spmd(
        _state["nc"], in_maps, core_ids=list(range(NCORES)), trace=trace
    )
    _state["last_results"] = res
    return np.concatenate([r["out"] for r in res.results], axis=0)



# revision 2
# speedup vs baseline: 1.0540x; 1.0540x over previous
"""Bezier2Image Trainium2 kernel (Bass/Tile, 8-core data parallel).

Computation per sample b:
  ctrl = x[b].reshape(160, 4, 2); pts = T @ ctrl  -> 4800 (curve, t) points
  gX[p, w] = exp(-(bX_w - X_p)^2 / ALPHA), gY likewise  (separable splat)
  out[b] = min(gX^T @ gY, 1)   (contraction over the 4800 points)

Device mapping (per core, 16 samples), v2 — ACT-walled design:
  - points in 40 chunks of 120 (4 curves x 30 samples), partition dim =
    point-within-chunk
  - nkxy[p, (c, t)] = -KS * pts: ONE fp32 matmul (Wc @ staged) into PSUM
  - ds[p, c, t, w] = KS*bX_w - KS*pts  (bf16): one broadcast tensor_tensor,
    split DVE (56 of 80 (c,t)-units) / GPSIMD (24 units) to keep both
    under the ACT wall; GPSIMD reads a DVE-copied SBUF mirror of nkxy
  - gaussians: ONE ACT pass per sample over [120, 4800]:
    Derivative_Erf(x) = (2/sqrt(pi)) * exp(-x^2); the (4/pi) factor on
    gX*gY is undone in the epilogue.  ACT busy = 16*(224+4800)/1.2 ~ 67us
    = the roofline for this kernel (only ACT can exp; 9.2M exps/core).
  - accumulation: 40 bf16 matmuls [120x60]^T @ [120x60] into one PSUM bank
  - epilogue on DVE: min(res * pi/4, 1), DMA out
"""

import numpy as np

N = 30
W = 60
LENGTH = 160
ALPHA = 2e-4
B = 128
NCORES = 8
BPC = B // NCORES  # samples per core
KS = float(1.0 / np.sqrt(ALPHA))
NCH = 40  # chunks per sample
PCH = 120  # points per chunk (4 curves x 30)
DVE_U = 56  # of the 80 (chunk, coord) ds units, how many DVE computes

_state = {}


def _bezier_T():
    t = np.arange(N, dtype=np.float64) / N
    t = 2.0 * t**3 - 3.0 * t**2 + 2.0 * t
    t3 = t**3
    T = np.stack(
        [t3, 3.0 * (t**2 - t3), 3.0 * (t3 - 2.0 * t**2 + t), (1.0 - t) ** 3],
        axis=1,
    )
    return T  # [N, 4] float64


def build_nc(loop_n=1, sim_safe=False):
    from contextlib import ExitStack

    import concourse.bacc as bacc
    import concourse.mybir as mybir
    import concourse.tile as tile

    fp32 = mybir.dt.float32
    bf16 = mybir.dt.bfloat16
    AF = mybir.ActivationFunctionType

    # Bacc (not plain Bass): its compile() pass splits multi-sem waits into
    # event-semaphore instructions — walrus codegen allows only one sync wait
    # per compute instruction.
    nc = bacc.Bacc()
    x_in = nc.declare_dram_parameter("x", [BPC, LENGTH, 8], fp32, isOutput=False)
    out_d = nc.declare_dram_parameter("out", [BPC, W, W], fp32, isOutput=True)

    # Constants.
    T = _bezier_T()  # [30, 4]
    q = np.arange(PCH)
    # Wc[(dl, k), q] = -KS * T[q % 30, k] if q // 30 == dl else 0.
    # One matmul Wc.T @ staged then computes -KS * pts for a whole
    # sample: nkXY[q, (c,t)] = sum_{dl,k} Wc[(dl,k), q] * x[b, 4c+dl, 2k+t].
    Wc_np = np.zeros((16, PCH), np.float32)
    for dl in range(4):
        for k in range(4):
            row = np.where(q // N == dl, -KS * T[q % N, k], 0.0)
            Wc_np[dl * 4 + k] = row.astype(np.float32)
    bxk_np = np.broadcast_to(
        (KS * np.arange(W, dtype=np.float64) / W).astype(np.float32), (128, W)
    ).copy()

    Wc_d = nc.inline_tensor(Wc_np, "Wc")
    bxk_d = nc.inline_tensor(bxk_np, "bxk")

    with ExitStack() as ctx:
        tc = ctx.enter_context(tile.TileContext(nc))
        consts = ctx.enter_context(tc.tile_pool(name="consts", bufs=1))
        small = ctx.enter_context(tc.tile_pool(name="small", bufs=4))
        big = ctx.enter_context(tc.tile_pool(name="big", bufs=3))
        psum = ctx.enter_context(tc.tile_pool(name="psum", bufs=3, space="PSUM"))
        psum_pts = ctx.enter_context(tc.tile_pool(name="psum_pts", bufs=3, space="PSUM"))
        outp = ctx.enter_context(tc.tile_pool(name="outp", bufs=6))

        Wc = consts.tile([16, PCH], fp32)
        nc.sync.dma_start(out=Wc, in_=Wc_d[:, :])
        bxk = consts.tile([128, W], fp32)
        nc.sync.dma_start(out=bxk, in_=bxk_d[:, :])

        loop_ctx = tc.For_i(0, loop_n, 1) if loop_n > 1 else None
        if loop_ctx is not None:
            ctx.enter_context(loop_ctx)

        for b in range(BPC):
            # staged[(dl,k), c, t] = x[b, 4c+dl, 2k+t]
            staged = small.tile([16, NCH, 2], fp32)
            xb = x_in[b].rearrange("(c dl) (k t) -> dl k c t", dl=4, t=2)
            for t in range(2):
                nc.sync.dma_start(
                    out=staged[:, :, t],
                    in_=xb[:, :, :, t].rearrange("dl k c -> (dl k) c"),
                )
            # nkxy[q, (c, t)] = -KS * pts[l(q,c), n(q), t]
            nkxy = psum_pts.tile([PCH, NCH * 2], fp32, name=f"nkxy_{b}", tag="nkxy")
            nc.tensor.matmul(nkxy, Wc, staged.rearrange("k c t -> k (c t)"))
            # SBUF mirror for GPSIMD (it cannot read PSUM).
            nkxy_sb = small.tile([PCH, NCH * 2], fp32, name=f"nkxysb_{b}", tag="nkxy_sb")
            nc.vector.tensor_copy(nkxy_sb, nkxy)

            # ds[q, u, w] = KS*bX_w - KS*pts  for the 80 (c,t) units u
            ds = big.tile([PCH, NCH * 2, W], bf16, name=f"ds_{b}", tag="ds")
            nc.vector.tensor_add(
                ds[:, :DVE_U],
                bxk[:PCH].unsqueeze(1).broadcast_to([PCH, DVE_U, W]),
                nkxy[:, :DVE_U].unsqueeze(2).broadcast_to([PCH, DVE_U, W]),
            )
            nc.gpsimd.tensor_add(
                ds[:, DVE_U:],
                bxk[:PCH].unsqueeze(1).broadcast_to([PCH, NCH * 2 - DVE_U, W]),
                nkxy_sb[:, DVE_U:].unsqueeze(2).broadcast_to([PCH, NCH * 2 - DVE_U, W]),
            )

            # One ACT pass: g = (2/sqrt(pi)) * exp(-ds^2)
            g = big.tile([PCH, NCH * 2, W], bf16, name=f"g_{b}", tag="g")
            if sim_safe:
                # CoreSim lacks Derivative_Erf: equivalent two-op path.
                d2 = big.tile([PCH, NCH * 2, W], bf16, name=f"d2_{b}", tag="d2")
                nc.vector.tensor_mul(d2, ds, ds)
                nc.scalar.activation(g, d2, AF.Exp, scale=-1.0)
                nc.vector.tensor_scalar_mul(g, g, float(2.0 / np.sqrt(np.pi)))
            else:
                nc.scalar.activation(g, ds, AF.Derivative_Erf)

            res = psum.tile([W, W], fp32)
            gv = g.rearrange("q (c t) w -> q c t w", t=2)
            for c in range(NCH):
                nc.tensor.matmul(
                    res,
                    gv[:, c, 0],
                    gv[:, c, 1],
                    start=(c == 0),
                    stop=(c == NCH - 1),
                )

            res_sb = outp.tile([W, W], fp32, name=f"rs_{b}", tag="res_sb")
            # res carries the (2/sqrt(pi))^2 factor from Derivative_Erf:
            # undo with *pi/4, then clamp.
            nc.vector.tensor_scalar(
                res_sb,
                res,
                float(np.pi / 4.0),
                1.0,
                op0=mybir.AluOpType.mult,
                op1=mybir.AluOpType.min,
            )
            nc.sync.dma_start(out=out_d[b], in_=res_sb)

    nc.compile()
    return nc


def kernel(x):
    import os

    x = np.ascontiguousarray(x, dtype=np.float32)
    assert x.shape == (B, LENGTH, 8), x.shape
    if "nc" not in _state:
        _state["nc"] = build_nc()
    from concourse.bass_utils import run_bass_kernel_spmd

    in_maps = [{"x": x[i * BPC : (i + 1) * BPC]} for i in range(NCORES)]
    trace = bool(os.environ.get("BEZIER_TRACE"))
    res = run_bass_kernel_spmd(
        _state["nc"], in_maps, core_ids=list(range(NCORES)), trace=trace
    )
    _state["last_results"] = res
    return np.concatenate([r["out"] for r in res.results], axis=0)
